# revision 26
# baseline (speedup 1.0000x reference)
"""LogitSeparator Trainium2 kernel.

For each (b, d) of schemas (64, 32), left-align the zone
logits[b, start:end] (length = schemas[b,d] <= 255) into out[b, d, :8192],
zero padded, plus a boolean in-zone mask.

Strategy: pure data parallel over the batch dim (8 rows per core).  Per
core the 256 ragged (b, d) rows map onto 128 SBUF partitions x 2 column
halves (row r = 2*p + h).  Two indirect DMAs (one per half) gather each
row's 256-element slab from the (padded, flat) logits in DRAM using
per-partition flat start offsets; a warm-up indirect DMA is issued
first so the SWDGE ucode cold start (~1us) overlaps the aux input DMA's
flight, and the aux wait rides on the h0 gather instruction itself.
The vector engine builds the j < len mask and zeroes the slab tail
garbage; half 1 (the critical tail) is split into quarters so its two
out DMAs can start earlier on the two HWDGE queues (sync + scalar).
Since every zone length is <= 255, columns 256..8191 of the full output
are structurally zero — the device only writes the informative 256-wide
slabs and the host unshard step places them into zero-filled full-shape
arrays.
"""

import numpy as np

import concourse.bass as bass
import concourse.mybir as mybir
from concourse.bass_utils import run_bass_kernel_spmd

B, D, L = 64, 32, 8192
NCORES = 8
BPC = B // NCORES           # batch rows per core
R = BPC * D                 # ragged rows per core (256)
P = 128                     # SBUF partitions
HALVES = R // P             # 2
SLAB = 256                  # max zone length (schemas < 256)
NPAD = BPC * L + SLAB       # padded flat logits length per core
W = HALVES * SLAB           # SBUF row width (512)

_NC_CACHE = {}

# aux layout (int32): cols [0:2] gather flat-start idx (row r = 2p + h),
# cols [2:4] zone lens.
AUXW = 2 * HALVES


def build_nc():
    nc = bass.Bass()
    lg = nc.declare_dram_parameter(
        "logits_flat", [NPAD, 1], mybir.dt.float32, isOutput=False
    )
    aux = nc.declare_dram_parameter("aux", [P, AUXW], mybir.dt.int32, isOutput=False)
    # Device outputs hold only the informative 256-col slabs, laid out
    # exactly like the SBUF tiles; the host de-interleaves (row r = 2p+h).
    out = nc.declare_dram_parameter("out", [P, W], mybir.dt.float32, isOutput=True)
    msk = nc.declare_dram_parameter("mask", [P, W], mybir.dt.uint8, isOutput=True)

    with (
        nc.sbuf_tensor([P, AUXW], mybir.dt.int32) as aux_t,
        nc.sbuf_tensor([P, SLAB], mybir.dt.int32) as iota_t,
        nc.sbuf_tensor([P, W], mybir.dt.float32) as gat2,
        nc.sbuf_tensor([P, W], mybir.dt.uint8) as masku2,
        nc.sbuf_tensor([P, 8], mybir.dt.float32) as wdst,
        nc.semaphore("asem") as asem,    # aux input DMA completion
        nc.semaphore("isem") as isem,    # iota generated
        nc.semaphore("wsem") as wsem,    # warm-up gather completion
        nc.semaphore("g0sem") as g0sem,  # gather h0 completion
        nc.semaphore("g1sem") as g1sem,  # gather h1 completion
        nc.semaphore("vsem") as vsem,    # mask compare done
        nc.semaphore("m0sem") as m0sem,  # mul h0 done
        nc.semaphore("m1asem") as m1asem,  # mul h1 first quarter done
        nc.semaphore("m1bsem") as m1bsem,  # mul h1 second quarter done
        nc.semaphore("dsem") as dsem,    # out h0 + h1b DMA completions
        nc.semaphore("esem") as esem,    # out h1a DMA completion
        nc.semaphore("msem") as msem,    # mask DMA completion
        nc.Block() as block,
    ):
        Q = SLAB // 2  # quarter width (128)

        @block.sync
        def _(sync):
            sync.dma_start(out=aux_t[:], in_=aux[:]).then_inc(asem, 16)
            sync.dma_start(
                out=out[:, 0:SLAB], in_=gat2[:, 0:SLAB]
            ).wait_op(m0sem, 1, "sem-ge").then_inc(dsem, 16)
            sync.dma_start(
                out=out[:, SLAB : SLAB + Q], in_=gat2[:, SLAB : SLAB + Q]
            ).wait_op(m1asem, 1, "sem-ge").then_inc(dsem, 16)
            sync.wait_ge(dsem, 32)

        @block.gpsimd
        def _(gp):
            # Index ramp 0..255 per partition, generated during the aux
            # DMA's flight; its first column (all zeros) doubles as the
            # warm-up's offset table.
            gp.iota(
                iota_t[:], pattern=[[1, SLAB]], base=0, channel_multiplier=0
            ).then_inc(isem, 1)
            # Warm-up: a throwaway indirect DMA so the SWDGE ucode cold
            # start (~1us) happens during the aux DMA's flight, not on the
            # critical path of the real gathers.
            gp.indirect_dma_start(
                out=wdst[:],
                out_offset=None,
                in_=lg[:],
                in_offset=bass.IndirectOffsetOnAxis(ap=iota_t[:, 0:1], axis=0),
            ).then_inc(wsem, 16)
            # One indirect gather per half: partition p of half h holds
            # ragged row r = 2p + h.  The aux wait is attached to the h0
            # gather so its SWDGE prep can overlap the warm-up's issue.
            gsems = [g0sem, g1sem]
            for h in range(HALVES):
                inst = gp.indirect_dma_start(
                    out=gat2[:, h * SLAB : (h + 1) * SLAB],
                    out_offset=None,
                    in_=lg[:],
                    in_offset=bass.IndirectOffsetOnAxis(
                        ap=aux_t[:, h : h + 1], axis=0
                    ),
                )
                if h == 0:
                    inst.wait_op(asem, 16, "sem-ge")
                inst.then_inc(gsems[h], 16)
            gp.wait_ge(wsem, 16)

        @block.vector
        def _(v):
            v.wait_ge(isem, 1)   # iota ramp ready
            v.wait_ge(asem, 16)  # aux (lens) in SBUF
            # mask[p, h, j] = j < len_ph  (int32 compare, uint8 0/1 out),
            # both halves in one op via dual broadcast.
            v.tensor_tensor(
                out=masku2[:].rearrange("p (h j) -> p h j", h=HALVES),
                in0=iota_t[:].unsqueeze(1).to_broadcast([P, HALVES, SLAB]),
                in1=aux_t[:, HALVES : 2 * HALVES]
                .unsqueeze(2)
                .to_broadcast([P, HALVES, SLAB]),
                op=mybir.AluOpType.is_lt,
            ).then_inc(vsem, 1)
            v.drain()  # flush DVE pipeline: masku2 RAW below
            # Zero the gathered tail garbage (j >= len) in place.  Half 1
            # (the critical tail) is split into quarters so its two out
            # DMAs can start earlier on the two HWDGE queues.
            v.tensor_mul(
                out=gat2[:, 0:SLAB],
                in0=gat2[:, 0:SLAB],
                in1=masku2[:, 0:SLAB],
            ).wait_op(g0sem, 16, "sem-ge").then_inc(m0sem, 1)
            v.tensor_mul(
                out=gat2[:, SLAB : SLAB + Q],
                in0=gat2[:, SLAB : SLAB + Q],
                in1=masku2[:, SLAB : SLAB + Q],
            ).wait_op(g1sem, 16, "sem-ge").then_inc(m1asem, 1)
            v.tensor_mul(
                out=gat2[:, SLAB + Q : W],
                in0=gat2[:, SLAB + Q : W],
                in1=masku2[:, SLAB + Q : W],
            ).then_inc(m1bsem, 1)

        @block.scalar
        def _(sc):
            # Mask slab (u8) only needs the compares (vsem >= 1).
            sc.dma_start(out=msk[:], in_=masku2[:]).wait_op(
                vsem, 1, "sem-ge"
            ).then_inc(msem, 16)
            sc.dma_start(
                out=out[:, SLAB + Q : W], in_=gat2[:, SLAB + Q : W]
            ).wait_op(m1bsem, 1, "sem-ge").then_inc(esem, 16)
            sc.wait_ge(msem, 16)
            sc.wait_ge(esem, 16)
    return nc


def _get_nc():
    if "nc" not in _NC_CACHE:
        _NC_CACHE["nc"] = build_nc()
    return _NC_CACHE["nc"]


def make_in_maps(schemas, logits):
    """Shard full inputs into per-core input maps for the SPMD kernel."""
    sch = np.asarray(schemas).astype(np.int64)
    lg = np.ascontiguousarray(np.asarray(logits, dtype=np.float32))
    cs = np.cumsum(sch, axis=1)
    start = cs - sch                     # (B, D) zone starts

    in_maps = []
    for c in range(NCORES):
        b0 = c * BPC
        flat = np.concatenate(
            [lg[b0 : b0 + BPC].reshape(-1), np.zeros(SLAB, np.float32)]
        ).reshape(NPAD, 1)
        gflat = (
            np.arange(BPC, dtype=np.int64)[:, None] * L + start[b0 : b0 + BPC]
        ).reshape(R)
        aux = np.empty((P, AUXW), dtype=np.int32)
        # row r = 2*p + h  ->  aux[p, h]
        aux[:, 0:HALVES] = gflat.reshape(P, HALVES)
        aux[:, HALVES : 2 * HALVES] = (
            sch[b0 : b0 + BPC].reshape(R).reshape(P, HALVES).astype(np.int32)
        )
        in_maps.append({"logits_flat": flat, "aux": aux})
    return in_maps


def assemble(results):
    """Gather per-core slab outputs into zero-filled full-shape arrays."""
    out = np.zeros((B, D, L), dtype=np.float32)
    msk = np.zeros((B, D, L), dtype=np.uint8)
    for c in range(NCORES):
        b0 = c * BPC
        out[b0 : b0 + BPC, :, :SLAB] = (
            np.asarray(results[c]["out"]).reshape(BPC, D, SLAB)
        )
        msk[b0 : b0 + BPC, :, :SLAB] = (
            np.asarray(results[c]["mask"]).reshape(BPC, D, SLAB)
        )
    return out, msk.view(np.bool_)


def kernel(schemas, logits):
    in_maps = make_in_maps(schemas, logits)
    nc = _get_nc()
    res = run_bass_kernel_spmd(nc, in_maps, list(range(NCORES))).results
    return assemble(res)
